# revision 30
# baseline (speedup 1.0000x reference)
"""Causal self-attention (B=4, T=2048, C=1024, H=16) on 8 TRN2 NeuronCores.

Sharding: 8 cores = 4 batches x 2 head-groups (Megatron tensor-parallel over
heads + data-parallel over batch). Each core computes, for its batch b and its
8 heads, a fused flash-style pipeline over 512-token tiles t:

  phase t: attention for query tile t (all 4 head pairs, causal key tiles)
           interleaved by the Tile scheduler with stage-1 qkv projection of
           tile t+1 and stage-3 output projection of tile t-1, so the PE
           fills the gaps where attention waits on the Scalar-engine exp.

Layouts (all matmul operands bf16; PSUM accumulation fp32):
  qT/kT [head_dim on partitions pair-packed: even head dims 0-63, odd 64-127]
  V [tokens on partitions, head-major dims on free] with an appended
    ones-column per head so the PV matmul computes softmax sums for free.
  exp on the Scalar engine straight out of PSUM (no max-subtraction: logits
  are O(1) by construction, 1/sqrt(hd) folded into w_q); the causal triangle
  of diagonal blocks is zeroed by a 0/1 bf16 mask multiply on the Vector
  engine after the exp, keeping the mask off the PE (the bottleneck engine).

Host: shards inputs (bf16), sums the two partial outputs per batch, adds b_proj.
"""

import sys

if "/opt/trn_rl_repo" not in sys.path:
    sys.path.insert(0, "/opt/trn_rl_repo")

from contextlib import ExitStack

import numpy as np
import ml_dtypes

import concourse.bass as bass
import concourse.tile as tile
from concourse import bacc, mybir
from concourse.bass_utils import run_bass_kernel_spmd

F32 = mybir.dt.float32
BF16 = mybir.dt.bfloat16
AF = mybir.ActivationFunctionType
ADD = mybir.AluOpType.add

B, T, C = 4, 2048, 1024
H, HD = 16, 64
NHL = 8          # heads per core (local)
NPAIR = 4        # head pairs per core
P = 128
TQ = 512         # query tile (free dim)
TJ = 128         # key tile (partitions)
NIT = T // TQ    # 4 query tiles
NTS = T // P     # 16 token sub-tiles
NCT = C // P     # 8 contraction tiles over C


def build_kernel():
    nc = bacc.Bacc("TRN2", target_bir_lowering=False)

    xt = nc.declare_dram_parameter("xt", [C, T], BF16, isOutput=False)
    wqk = nc.declare_dram_parameter("wqk", [P, NCT, 1024], BF16, isOutput=False)
    bqk = nc.declare_dram_parameter("bqk", [P, 8], F32, isOutput=False)
    wv = nc.declare_dram_parameter("wv", [P, NCT, 512], BF16, isOutput=False)
    bv = nc.declare_dram_parameter("bv", [1, 512], F32, isOutput=False)
    wp = nc.declare_dram_parameter("wp", [P, NPAIR, 1024], BF16, isOutput=False)
    tri = nc.declare_dram_parameter("tri", [P, 2, P], BF16, isOutput=False)
    out = nc.declare_dram_parameter("out", [T, C], F32, isOutput=True)

    with tile.TileContext(nc) as tc, ExitStack() as ctx:
        persist = ctx.enter_context(tc.tile_pool(name="persist", bufs=1))
        xp = ctx.enter_context(tc.tile_pool(name="xp", bufs=4))
        attp = ctx.enter_context(tc.tile_pool(name="attp", bufs=8))
        rtp = ctx.enter_context(tc.tile_pool(name="rtp", bufs=6))
        rbp = ctx.enter_context(tc.tile_pool(name="rbp", bufs=6))
        otp = ctx.enter_context(tc.tile_pool(name="otp", bufs=3))
        accps = ctx.enter_context(tc.tile_pool(name="accps", bufs=2, space="PSUM"))
        qkps = ctx.enter_context(tc.tile_pool(name="qkps", bufs=2, space="PSUM"))
        pvps = ctx.enter_context(tc.tile_pool(name="pvps", bufs=2, space="PSUM"))

        q_sb = persist.tile([P, NPAIR, T], BF16)
        k_sb = persist.tile([P, NPAIR, T], BF16)
        v_sb = persist.tile([P, NTS, NHL, HD + 1], BF16)
        y_sb = persist.tile([P, NPAIR, T], BF16)
        wqk_sb = persist.tile([P, NCT, 1024], BF16)
        wv_sb = persist.tile([P, NCT, 512], BF16)
        wp_sb = persist.tile([P, NPAIR, 1024], BF16)
        bqk_sb = persist.tile([P, 8], F32)
        bv_sb = persist.tile([P, 512], F32)
        tri_sb = persist.tile([P, 2, P], BF16)

        def s1_load(t, split=False):
            # one DMA per 512-token x tile: [C_chunk on partitions, chunk, tok]
            t0 = t * TQ
            xi = xp.tile([P, NCT, TQ], BF16, tag="xc")
            src = xt[:, t0 : t0 + TQ].rearrange("(c p) t -> p c t", p=P)
            if split:
                # two DMAs so the first c-chunks land sooner at kernel start
                nc.sync.dma_start(xi[:, 0:4, :], src[:, 0:4, :])
                nc.sync.dma_start(xi[:, 4:8, :], src[:, 4:8, :])
            else:
                nc.sync.dma_start(xi, src)
            return xi

        def s1_compute(t, xi):
            t0 = t * TQ
            # q (m 0-3) and k (m 4-7) blocks: out [f-part, t-free]
            for m in range(8):
                ps = accps.tile([P, TQ], F32, tag="acc")
                for c in range(NCT):
                    nc.tensor.matmul(
                        ps,
                        wqk_sb[:, c, m * P : (m + 1) * P],
                        xi[:, c, :],
                        start=(c == 0),
                        stop=(c == NCT - 1),
                    )
                dst = q_sb if m < 4 else k_sb
                nc.vector.tensor_scalar_add(
                    dst[:, m % 4, t0 : t0 + TQ], ps, bqk_sb[:, m : m + 1]
                )
            # v blocks: out [t-part, f-free(head-major)]
            for s in range(TQ // P):
                ps = accps.tile([P, 512], F32, tag="acc")
                for c in range(NCT):
                    nc.tensor.matmul(
                        ps,
                        xi[:, c, s * P : (s + 1) * P],
                        wv_sb[:, c, :],
                        start=(c == 0),
                        stop=(c == NCT - 1),
                    )
                tsub = t * (TQ // P) + s
                nc.vector.tensor_tensor(
                    v_sb[:, tsub, :, 0:HD],
                    ps.rearrange("p (h d) -> p h d", h=NHL),
                    bv_hd,
                    ADD,
                )

        def att_phase(it):
            i0 = it * TQ
            njt = (i0 + TQ) // TJ
            for a in range(NPAIR):
                pv0 = pvps.tile([P, TQ], F32, tag="pv", name=f"pv0_{a}_{it}")
                pv1 = pvps.tile([P, TQ], F32, tag="pv", name=f"pv1_{a}_{it}")
                for jt in range(njt):
                    j0 = jt * TJ
                    d = j0 - i0
                    istart = max(d, 0)
                    nn = TQ - istart
                    qk = qkps.tile([P, 2, TQ], F32, tag="qk")
                    for e in (0, 1):
                        nc.tensor.matmul(
                            qk[:, e, istart:TQ],
                            k_sb[64 * e : 64 * e + 64, a, j0 : j0 + TJ],
                            q_sb[64 * e : 64 * e + 64, a, i0 + istart : i0 + TQ],
                            start=True,
                            stop=True,
                            tile_position=(64 * e, 0),
                        )
                    att = attp.tile([P, 2, TQ], BF16, tag="att")
                    nc.scalar.activation(att[:, :, 0:nn], qk[:, :, istart:TQ], AF.Exp)
                    if d >= 0:
                        # zero the upper triangle of the diagonal block (the
                        # first TJ query columns) off the PE's critical path
                        nc.vector.tensor_tensor(
                            att[:, :, 0:TJ], att[:, :, 0:TJ], tri_sb,
                            mybir.AluOpType.mult,
                        )
                    last = jt == njt - 1
                    nc.tensor.matmul(
                        pv0[0 : HD + 1, istart:TQ],
                        v_sb[:, jt, 2 * a, :],
                        att[:, 0, 0:nn],
                        start=(jt == 0),
                        stop=last,
                    )
                    nc.tensor.matmul(
                        pv1[0 : HD + 1, istart:TQ],
                        v_sb[:, jt, 2 * a + 1, :],
                        att[:, 1, 0:nn],
                        start=(jt == 0),
                        stop=last,
                    )
                # normalize: softmax sums sit at row HD of each pv tile
                rt = rtp.tile([P, TQ], F32, tag="rt")
                rb = rbp.tile([P, TQ], F32, tag="rb")
                nc.vector.reciprocal(rt[HD : HD + 1, :], pv0[HD : HD + 1, :])
                nc.sync.dma_start(rb[0:1, :], rt[HD : HD + 1, :])
                nc.gpsimd.partition_broadcast(rb[0:HD, :], rb[0:1, :])
                nc.vector.tensor_mul(
                    y_sb[0:HD, a, i0 : i0 + TQ], pv0[0:HD, :], rb[0:HD, :]
                )
                rt1 = rtp.tile([P, TQ], F32, tag="rt")
                rb1 = rbp.tile([P, TQ], F32, tag="rb")
                nc.vector.reciprocal(rt1[HD : HD + 1, :], pv1[HD : HD + 1, :])
                nc.sync.dma_start(rb1[0:1, :], rt1[HD : HD + 1, :])
                nc.gpsimd.partition_broadcast(rb1[0:HD, :], rb1[0:1, :])
                yt = rtp.tile([P, TQ], BF16, tag="yt")
                nc.vector.tensor_mul(yt[0:HD, :], pv1[0:HD, :], rb1[0:HD, :])
                nc.sync.dma_start(y_sb[HD:P, a, i0 : i0 + TQ], yt[0:HD, :])

        def s3_tile(tt, fine=False):
            ot_sb = otp.tile([P, 1024], F32, tag="osb")
            for ot in range(2):
                ps = accps.tile([P, 512], F32, tag="acc")
                for a in range(NPAIR):
                    nc.tensor.matmul(
                        ps,
                        y_sb[:, a, tt * P : (tt + 1) * P],
                        wp_sb[:, a, ot * 512 : (ot + 1) * 512],
                        start=(a == 0),
                        stop=(a == NPAIR - 1),
                    )
                nc.vector.tensor_copy(ot_sb[:, ot * 512 : (ot + 1) * 512], ps)
                if fine:
                    # last tiles: ship each half as soon as its copy lands so
                    # the final copy->DMA chain is half as long
                    nc.sync.dma_start(
                        out[tt * P : (tt + 1) * P, ot * 512 : (ot + 1) * 512],
                        ot_sb[:, ot * 512 : (ot + 1) * 512],
                    )
            if not fine:
                nc.sync.dma_start(out[tt * P : (tt + 1) * P, :], ot_sb)

        # ---------------- fused pipeline ----------------
        # head: x(0) + weights interleaved finely — the DMA device transfers
        # in issue order, and the first m-chunk accumulations need the low-c
        # x/wqk chunks first
        x0 = xp.tile([P, NCT, TQ], BF16, tag="xc")
        src0 = xt[:, 0:TQ].rearrange("(c p) t -> p c t", p=P)
        nc.sync.dma_start(x0[:, 0:4, :], src0[:, 0:4, :])
        nc.sync.dma_start(wqk_sb[:, 0:2, :], wqk[:, 0:2, :])
        nc.sync.dma_start(wqk_sb[:, 2:4, :], wqk[:, 2:4, :])
        nc.sync.dma_start(x0[:, 4:8, :], src0[:, 4:8, :])
        nc.sync.dma_start(wqk_sb[:, 4:6, :], wqk[:, 4:6, :])
        nc.sync.dma_start(wqk_sb[:, 6:8, :], wqk[:, 6:8, :])
        nc.sync.dma_start(wv_sb, wv[:])
        nc.sync.dma_start(bqk_sb, bqk[:])
        nc.sync.dma_start(tri_sb, tri[:])
        nc.sync.dma_start(bv_sb[0:1, :], bv[:])
        nc.gpsimd.partition_broadcast(bv_sb[:, :], bv_sb[0:1, :])
        bv_hd = bv_sb.rearrange("p (h d) -> p h d", h=NHL)
        # ones columns of the augmented V
        nc.vector.memset(v_sb[:, :, :, HD : HD + 1], 1.0)

        s1_compute(0, x0)
        nc.sync.dma_start(wp_sb, wp[:])
        xs = {}
        for t in range(NIT):
            if t + 1 < NIT:
                xs[t + 1] = s1_load(t + 1)
            att_phase(t)
            if t + 1 < NIT:
                s1_compute(t + 1, xs[t + 1])
        # stage 3 last: it back-fills the PE during the ACT-bound final
        # attention phase (the scheduler pulls it whenever attention stalls)
        for tt in range(NTS):
            s3_tile(tt, fine=(tt >= NTS - 2))

    nc.compile()
    return nc


_NC_CACHE = None


def _get_nc():
    global _NC_CACHE
    if _NC_CACHE is None:
        _NC_CACHE = build_kernel()
    return _NC_CACHE


def _shard_inputs(x, w_qkv, b_qkv, w_proj):
    """Build the 8 per-core input maps. Core id = 2*batch + head_group."""
    bf = ml_dtypes.bfloat16
    tri01 = np.where(
        np.arange(P)[None, :] >= np.arange(P)[:, None], 1.0, 0.0
    )
    tri_np = np.ascontiguousarray(
        np.stack([tri01, tri01], axis=1)
    ).astype(bf)  # [P, 2, P]

    in_maps = []
    for b in range(B):
        xt = np.ascontiguousarray(x[b].T).astype(bf)  # [C, T]
        for g in range(2):
            s = slice(g * 512, (g + 1) * 512)
            wqk_full = np.concatenate(
                [w_qkv[0:1024][s] / 8.0, w_qkv[1024:2048][s]], axis=0
            )  # [1024 f, 1024 c]
            wqk_arr = np.ascontiguousarray(
                wqk_full.T.reshape(NCT, P, 1024).transpose(1, 0, 2)
            ).astype(bf)
            bqk_full = np.concatenate([b_qkv[0:1024][s] / 8.0, b_qkv[1024:2048][s]])
            bqk_arr = np.ascontiguousarray(bqk_full.reshape(8, P).T)
            wv_rows = w_qkv[2048:3072][s]  # [512 f, 1024 c]
            wv_arr = np.ascontiguousarray(
                wv_rows.T.reshape(NCT, P, 512).transpose(1, 0, 2)
            ).astype(bf)
            bv_arr = np.ascontiguousarray(b_qkv[2048:3072][s][None, :])
            wp_rhs = w_proj[:, s].T  # [512 hd, 1024 o]
            wp_arr = np.ascontiguousarray(
                wp_rhs.reshape(NPAIR, P, 1024).transpose(1, 0, 2)
            ).astype(bf)
            in_maps.append(
                {
                    "xt": xt,
                    "wqk": wqk_arr,
                    "bqk": bqk_arr.astype(np.float32),
                    "wv": wv_arr,
                    "bv": bv_arr.astype(np.float32),
                    "wp": wp_arr,
                    "tri": tri_np,
                }
            )
    return in_maps


def kernel(x, w_qkv, b_qkv, w_proj, b_proj, _trace=False, _trace_kwargs=None):
    x = np.asarray(x, dtype=np.float32)
    w_qkv = np.asarray(w_qkv, dtype=np.float32)
    b_qkv = np.asarray(b_qkv, dtype=np.float32)
    w_proj = np.asarray(w_proj, dtype=np.float32)
    b_proj = np.asarray(b_proj, dtype=np.float32)

    nc = _get_nc()
    in_maps = _shard_inputs(x, w_qkv, b_qkv, w_proj)
    res = run_bass_kernel_spmd(
        nc, in_maps, core_ids=list(range(8)), trace=_trace,
        **(_trace_kwargs or {}),
    )
    out = np.empty((B, T, C), np.float32)
    for b in range(B):
        out[b] = res.results[2 * b]["out"] + res.results[2 * b + 1]["out"] + b_proj
    if _trace:
        return out, res
    return out
